# revision 5
# baseline (speedup 1.0000x reference)
"""Trainium2 Bass kernel for BiaffinePairing.

Computes S = (T @ W) @ A^T + T @ U[:H] + (A @ U[H:]).T + b  -> [4096, 4096] f32.

Strategy (8 NeuronCores, data-parallel over T's row dim n):
  - Host-side layout prep only (no math): transpose T and A so the
    contraction dim H=1024 lies on SBUF partitions; shard T^T's columns
    (the n dim) 8 ways; replicate A^T, W, and the U halves. Matmul inputs
    are pre-cast to fp16 on the host; the output is stored fp16 and
    upcast on the host (adds ~2e-4 to a 3.7e-4 relative error, halves
    the store stream).
  - Per core: mm1 computes TWt[h_out, n] = (T_shard @ W)^T accumulating in
    PSUM; the rank-1 term 1_n (x) (A @ u_a)^T folds in by adding u_a[h] as
    a per-partition bias on mm1's PSUM->SBUF copy (since
    (TW + 1 (x) u_a^T) @ A^T = TW@A^T + 1 (x) (A@u_a)^T).
  - tvec[n] = T_shard @ u_t + b via tiny matmuls; added as the per-partition
    bias on mm2's PSUM->SBUF copies.
  - mm2 computes S_shard[n, m] = sum_k TWt[k]^T @ At[k] over m-chunks.

Schedule notes (why the structure looks the way it does):
  - The ~6.6us framework preamble blocks every engine queue; the PE HAM
    clock-gate then needs ~5.7us of continuous busy to reach 8/8. A short
    memset-fed warmup run bridges only the DMA-arrival window (~2us) and
    mm1 itself rides through the ramp at half clock -- half-clock work is
    still work, unlike idle warmups.
  - Critical loads are spread over FOUR DMA queues (sync/scalar/gpsimd/
    vector issue rings) in exact consumption order, with W k-tiles split
    into front (ho 0..3) and back halves so mm1's pass A starts sooner.
  - mm1 uses all 8 PSUM banks (single pool, bufs=8): pass B never waits
    on pass A's copy-outs, and mm2's first accumulation starts right
    after mm1's last matmul (tvec's 2.2us covers the copy drain).
  - No scalar-engine compute anywhere: copies ride vector/gpsimd, so the
    framework emits no ACT_TABLE_LOAD and the scalar queue issues its
    first DMA ~1us earlier.
  - The very last output tile's copy and store are split across two
    engines / two queues to shorten the drain tail.
"""

import numpy as np

import concourse.bacc as bacc
import concourse.mybir as mybir
from concourse.tile import TileContext
from concourse.bass_utils import run_bass_kernel_spmd

H = 1024          # hidden dim (contraction)
N_TOT = 4096      # rows of target_spans
M_TOT = 4096      # rows of argument_spans
N_CORES = 8
NSH = N_TOT // N_CORES   # 512 n rows per core
KT = H // 128            # 8 contraction k-tiles
NI = NSH // 128          # 4 n-tiles of 128 per core
MCH = 1024               # m-chunk width
MC = M_TOT // MCH        # 4 m-chunks
MH = MCH // 512          # 512-wide psum sub-slices per chunk

F32 = mybir.dt.float32
F16 = mybir.dt.float16

_NC_CACHE = {}


def _build(b_val: float, warm: int = 9, warm_w: int = 256):
    nc = bacc.Bacc("TRN2", target_bir_lowering=False, debug=False,
                   num_devices=N_CORES)

    tT = nc.dram_tensor("tT", [H, NSH], F16, kind="ExternalInput")
    aT = nc.dram_tensor("aT", [H, M_TOT], F16, kind="ExternalInput")
    W = nc.dram_tensor("W", [H, H], F16, kind="ExternalInput")
    # ut comes in as [H, 2] (two identical columns -> a 2-wide moving
    # operand; both result columns equal tvec).
    ut = nc.dram_tensor("ut", [H, 2], F16, kind="ExternalInput")
    ua = nc.dram_tensor("ua", [H, 1], F32, kind="ExternalInput")
    out = nc.dram_tensor("out", [NSH, M_TOT], F16, kind="ExternalOutput")

    # DRAM views with the k-tile index split out: row kt*128 + p.
    tT_v = tT.rearrange("(kt p) n -> p kt n", p=128)
    aT_v = aT.rearrange("(kt p) m -> p kt m", p=128)
    W_v = W.rearrange("(kt p) f -> p kt f", p=128)
    ut_v = ut.rearrange("(kt p) two -> p kt two", p=128)
    ua_v = ua.rearrange("(kt p) one -> p (kt one)", p=128)

    with TileContext(nc) as tc:
        with (
            tc.tile_pool(name="const", bufs=1) as cpool,
            tc.tile_pool(name="achunk", bufs=4) as apool,
            tc.tile_pool(name="outbuf", bufs=6) as opool,
            tc.tile_pool(name="ps", bufs=8, space="PSUM") as pspool,
        ):
            w_sb = cpool.tile([128, KT, H], F16, tag="w")
            tT_sb = cpool.tile([128, KT, NSH], F16, tag="tT")
            ua_sb = cpool.tile([128, KT], F32, tag="ua")
            ut_sb = cpool.tile([128, KT, 2], F16, tag="ut")
            warm_sb = cpool.tile([128, warm_w], F16, tag="warm")
            at_sb = [apool.tile([128, KT, MCH], F16, tag="at",
                                name=f"at{c}")
                     for c in range(MC)]

            # ---- warm tile memset on vector (earliest-free engine),
            # before vector's DMA issues so the PE ramp starts ASAP ----
            nc.vector.memset(warm_sb[:], 0.0)

            # ---- load DMAs: 3 issue queues (sync/scalar/gpsimd; vector
            # cannot issue DMAs), consumption order. Pass A of mm1
            # consumes (W[k] front half, tT[k]) in k order: sync and
            # gpsimd alternate the W fronts, scalar streams tT. ----
            for k in range(0, KT, 2):
                nc.sync.dma_start(out=w_sb[:, k, 0:512],
                                  in_=W_v[:, k, 0:512])
                nc.gpsimd.dma_start(out=w_sb[:, k + 1, 0:512],
                                    in_=W_v[:, k + 1, 0:512])
            for k in range(KT):
                nc.scalar.dma_start(out=tT_sb[:, k, :], in_=tT_v[:, k, :])
            # back halves of W (pass B)
            for k in range(0, KT, 2):
                nc.sync.dma_start(out=w_sb[:, k, 512:1024],
                                  in_=W_v[:, k, 512:1024])
                nc.gpsimd.dma_start(out=w_sb[:, k + 1, 512:1024],
                                    in_=W_v[:, k + 1, 512:1024])
            # ua/ut late: needed only at mm1 copy-out / tvec
            nc.scalar.dma_start(out=ua_sb[:], in_=ua_v[:])
            nc.scalar.dma_start(out=ut_sb[:], in_=ut_v[:])
            # at chunk 0 on scalar (lands ~15us, mm2 starts ~26us);
            # chunks 1-3 on sync behind the W halves.
            nc.scalar.dma_start(out=at_sb[0][:], in_=aT_v[:, :, 0:MCH])
            for c in range(1, MC):
                nc.sync.dma_start(out=at_sb[c][:],
                                  in_=aT_v[:, :, c * MCH:(c + 1) * MCH])

            # ---- PE warmup: bridge the DMA-arrival window (~2us) so the
            # HAM busy-clock starts ticking; mm1 then rides the tail of
            # the half-clock ramp doing real work. ----
            wps = pspool.tile([128, 512], F32, tag="ps", name="warm_ps")
            for _ in range(warm):
                nc.tensor.matmul(wps[:, 0:warm_w], warm_sb[:, 0:128],
                                 warm_sb[:], start=True, stop=True)

            # ---- mm1: TWt[h_out, n] = (T @ W)^T, + u_a bias on copy-out.
            # Pass A = ho 0..3 (front W halves) into banks 1-4, pass B =
            # ho 4..7 into banks 5-8: no copy-wait between passes. ----
            twt_sb = cpool.tile([128, KT, NSH], F16, tag="twt")
            mm1_ps = [pspool.tile([128, NSH], F32, tag="ps",
                                  name=f"mm1_{j}")
                      for j in range(8)]
            for half in range(2):
                for k in range(KT):
                    for j in range(4):
                        ho = half * 4 + j
                        nc.tensor.matmul(
                            mm1_ps[ho][:],
                            w_sb[:, k, ho * 128:(ho + 1) * 128],
                            tT_sb[:, k, :],
                            start=(k == 0),
                            stop=(k == KT - 1),
                        )
                for j in range(4):
                    ho = half * 4 + j
                    # PSUM reads are DVE/ACT-only; ACT is avoided (its
                    # table load delays the scalar DMA queue), so all
                    # copy-outs ride vector.
                    nc.vector.tensor_scalar_add(
                        out=twt_sb[:, ho, :], in0=mm1_ps[ho][:],
                        scalar1=ua_sb[:, ho:ho + 1],
                    )

            # ---- tvec[n] = T @ u_t + b: 32 tiny matmuls (ut is the
            # 2-wide moving operand; psum column 0 is tvec). Runs right
            # after mm1 so its ~2.5us of PE time covers the pass-B
            # copy-out drain before mm2 needs those banks. ----
            tvec_sb = cpool.tile([128, NI], F32, tag="tvec")
            for ni in range(NI):
                psv = pspool.tile([128, 512], F32, tag="ps", name="psv")
                for k in range(KT):
                    nc.tensor.matmul(
                        psv[:, 0:2],
                        tT_sb[:, k, ni * 128:(ni + 1) * 128],
                        ut_sb[:, k, :],
                        start=(k == 0),
                        stop=(k == KT - 1),
                    )
                nc.vector.tensor_scalar_add(
                    out=tvec_sb[:, ni:ni + 1], in0=psv[:, 0:1],
                    scalar1=float(b_val),
                )

            # ---- mm2: S[n, m] = sum_k TWt[k]^T @ At[k], + tvec bias ----
            for c in range(MC):
                for ni in range(NI):
                    for h in range(MH):
                        ps = pspool.tile([128, 512], F32, tag="ps",
                                         name="mm2")
                        for k in range(KT):
                            nc.tensor.matmul(
                                ps[:],
                                twt_sb[:, k, ni * 128:(ni + 1) * 128],
                                at_sb[c][:, k, h * 512:(h + 1) * 512],
                                start=(k == 0),
                                stop=(k == KT - 1),
                            )
                        o_sb = opool.tile([128, 512], F16, tag="o")
                        rows = slice(ni * 128, (ni + 1) * 128)
                        col0 = c * MCH + h * 512
                        last = (c == MC - 1 and ni == NI - 1
                                and h == MH - 1)
                        if not last:
                            nc.vector.tensor_scalar_add(
                                out=o_sb[:], in0=ps[:],
                                scalar1=tvec_sb[:, ni:ni + 1],
                            )
                            nc.scalar.dma_start(
                                out=out[rows, col0:col0 + 512],
                                in_=o_sb[:],
                            )
                        else:
                            # final tile: one copy, but the store split
                            # across two queues to shorten the drain tail
                            nc.vector.tensor_scalar_add(
                                out=o_sb[:], in0=ps[:],
                                scalar1=tvec_sb[:, ni:ni + 1],
                            )
                            nc.scalar.dma_start(
                                out=out[rows, col0:col0 + 256],
                                in_=o_sb[:, 0:256],
                            )
                            nc.sync.dma_start(
                                out=out[rows, col0 + 256:col0 + 512],
                                in_=o_sb[:, 256:512],
                            )

    nc.compile()
    return nc


def _get_nc(b_val: float):
    key = float(b_val)
    if key not in _NC_CACHE:
        _NC_CACHE[key] = _build(key)
    return _NC_CACHE[key]


def make_in_maps(target_spans, argument_spans, W, U, b):
    """Host-side layout prep: shard/transpose/cast the full inputs into the
    per-core input maps. Returns (in_maps, b_val)."""
    target_spans = np.asarray(target_spans, dtype=np.float32)
    argument_spans = np.asarray(argument_spans, dtype=np.float32)
    W = np.ascontiguousarray(np.asarray(W, dtype=np.float16))
    U = np.asarray(U, dtype=np.float32).reshape(2 * H, 1)
    b_val = float(np.asarray(b).reshape(-1)[0])

    tT = np.ascontiguousarray(target_spans.T.astype(np.float16))  # [H, N_TOT]
    aT = np.ascontiguousarray(argument_spans.T.astype(np.float16))  # [H, M_TOT]
    ut = np.ascontiguousarray(
        np.repeat(U[:H], 2, axis=1).astype(np.float16))  # [H, 2]
    ua = np.ascontiguousarray(U[H:])

    in_maps = [
        {
            "tT": np.ascontiguousarray(tT[:, i * NSH:(i + 1) * NSH]),
            "aT": aT,
            "W": W,
            "ut": ut,
            "ua": ua,
        }
        for i in range(N_CORES)
    ]
    return in_maps, b_val


def kernel(target_spans, argument_spans, W, U, b):
    in_maps, b_val = make_in_maps(target_spans, argument_spans, W, U, b)
    nc = _get_nc(b_val)
    res = run_bass_kernel_spmd(nc, in_maps, core_ids=list(range(N_CORES)))
    out = np.concatenate(
        [res.results[i]["out"] for i in range(N_CORES)], axis=0
    )
    return out.astype(np.float32, copy=False)
